# revision 1
# baseline (speedup 1.0000x reference)
"""EdgeFeatureRGCN Trainium2 kernel: 3-layer RGCN (basis decomposition, mean
aggregation per (dst, relation)) + BatchNorm + LeakyReLU + final L2 normalize.

Strategy (8 NeuronCores, SPMD):
  - Destination-range sharding: core c owns dst nodes [c*NLOC, (c+1)*NLOC).
  - Edges sorted by dst, packed into 128-edge chunks bound to static 64-dst
    windows.  Per chunk: indirect-DMA gather of the 128 src rows from a
    replicated DRAM node table (bf16), then a selection matmul
    accT[feat, 4 bases x 64 dst] += G[128e, feat]^T-style PSUM accumulation
    with host-precomputed per-edge basis weights folded into the selection
    matrix (w_eb = comp[et, b] / cnt(dst, et)).
  - Per 128-dst tile: 4 basis matmuls (accT @ bases_b) + root term + (L3) bias.
  - BN stats (sum, sumsq) via ones-matmul, AllReduce'd; scale/shift applied
    fused with leaky ReLU.  Next-layer node table rebuilt via AllGather (bf16).
  - Final layer: row L2-normalize, per-core slab output, host concat.
"""
import sys
sys.path.insert(0, "/opt/trn_rl_repo")
import numpy as np
import ml_dtypes

from concourse import bass, mybir, bacc, tile
from concourse.masks import make_identity

# problem constants (hardcoded per harness contract)
N, E, R, NB = 50000, 800000, 8, 4
IN, H, OUT = 64, 128, 64
BN_EPS = 1e-5
NCORE = 8
P = 128
WIN = 64                      # dst nodes per selection window
NLOC = N // NCORE             # 6250 real dst nodes per core
NT = (NLOC + P - 1) // P      # 49 transform tiles per core
NLOCP = NT * P                # 6272 padded local slab rows
NWIN = NLOCP // WIN           # 98 windows
NPAD = NCORE * NLOCP          # padded table rows (50176)
SELW = NB * WIN               # selection matrix columns per chunk

F32 = mybir.dt.float32
BF16 = mybir.dt.bfloat16
BF = ml_dtypes.bfloat16
DBG = None
DT = BF16          # on-device data dtype for tables/weights/selection
NPDT = BF


def _set_dims(n, e):
    """Testing hook: rescale the problem (keeps IN/H/OUT/R/NB)."""
    global N, E, NLOC, NT, NLOCP, NWIN, NPAD
    N, E = n, e
    NLOC = N // NCORE
    NT = (NLOC + P - 1) // P
    NLOCP = NT * P
    NWIN = NLOCP // WIN
    NPAD = NCORE * NLOCP


def _row_id(node):
    """global node id -> padded table row id"""
    return (node // NLOC) * NLOCP + (node % NLOC)


def host_prep(edge_index, edge_type, comps):
    """Build per-core gather-index and selection-matrix arrays.

    comps: list of 3 [R, NB] arrays.
    Returns (quota [NWIN] list, per_core list of dicts with idx/sel1..3).
    """
    src = np.asarray(edge_index[0], dtype=np.int64)
    dst = np.asarray(edge_index[1], dtype=np.int64)
    et = np.asarray(edge_type, dtype=np.int64)
    seg = dst * R + et
    cnt = np.bincount(seg, minlength=N * R)
    norm = 1.0 / np.maximum(cnt[seg], 1.0)

    core_of = dst // NLOC
    per_core_edges = []
    nchunks = np.zeros((NCORE, NWIN), dtype=np.int64)
    for c in range(NCORE):
        m = core_of == c
        ldst = (dst[m] - c * NLOC).astype(np.int64)
        order = np.argsort(ldst, kind="stable")
        ldst = ldst[order]
        es, ee, en = src[m][order], et[m][order], norm[m][order]
        w = ldst // WIN
        wc = np.bincount(w, minlength=NWIN)
        nchunks[c] = (wc + 127) // 128
        per_core_edges.append((ldst, es, ee, en, w, wc))

    quota = np.maximum(nchunks.max(axis=0), 1)
    base = np.concatenate([[0], np.cumsum(quota)])
    totch = int(base[-1])

    per_core = []
    for c in range(NCORE):
        ldst, es, ee, en, w, wc = per_core_edges[c]
        # position of each edge within its window group
        wstart = np.concatenate([[0], np.cumsum(wc)])
        j = np.arange(len(ldst)) - wstart[w]          # rank within window
        g = base[w] + j // 128                        # global chunk id
        s = j % 128                                   # slot within chunk
        col = ldst - w * WIN                          # dst column in window

        idx = np.zeros((totch, P), dtype=np.int32)
        idx[g, s] = _row_id(es).astype(np.int32)
        d = {"idx": idx.reshape(-1)}
        for li, comp in enumerate(comps):
            sel = np.zeros((totch, P, SELW), dtype=NPDT)
            wv = (np.asarray(comp, np.float32)[ee] * en[:, None]).astype(NPDT)
            for b in range(NB):
                sel[g, s, b * WIN + col] = wv[:, b]
            d[f"sel{li + 1}"] = sel
        per_core.append(d)
    return [int(q) for q in quota], totch, per_core


def build_program(quota, totch):
    base = np.concatenate([[0], np.cumsum(quota)])
    nc = bacc.Bacc("TRN2", target_bir_lowering=False, debug=False,
                   num_devices=NCORE)

    xtab = nc.dram_tensor("xtab", [NPAD, P], DT, kind="ExternalInput")
    xT_d = nc.dram_tensor("xT", [IN, NLOCP], DT, kind="ExternalInput")
    idx_d = nc.dram_tensor("idx", [totch * P], mybir.dt.int32,
                           kind="ExternalInput")
    sel_d = [nc.dram_tensor(f"sel{l}", [totch, P, SELW], DT,
                            kind="ExternalInput") for l in (1, 2, 3)]
    bas_d = [nc.dram_tensor(f"bases{l}", [P, NB * P], DT,
                            kind="ExternalInput") for l in (1, 2, 3)]
    root_d = [nc.dram_tensor(f"root{l}", [P, P], DT, kind="ExternalInput")
              for l in (1, 2, 3)]
    gb_d = [nc.dram_tensor(f"gb{l}", [1, 2 * H], F32, kind="ExternalInput")
            for l in (1, 2)]
    bias3_d = nc.dram_tensor("bias3", [1, OUT], DT, kind="ExternalInput")
    out_d = nc.dram_tensor("out", [NLOCP, OUT], F32, kind="ExternalOutput")

    inW = [IN, H, H]
    outW = [H, H, OUT]

    with tile.TileContext(nc) as tc:
        with tc.tile_pool(name="sb", bufs=1) as sbp, \
             tc.tile_pool(name="sbl", bufs=2) as sbl, \
             tc.tile_pool(name="psA", bufs=2, space="PSUM") as psA, \
             tc.tile_pool(name="psB", bufs=2, space="PSUM") as psB, \
             tc.tile_pool(name="psC", bufs=1, space="PSUM") as psC, \
             tc.tile_pool(name="psS", bufs=1, space="PSUM") as psS, \
             tc.tile_pool(name="dram", bufs=1, space="DRAM") as drp:

            ident = sbp.tile([P, P], DT, tag="ident")
            make_identity(nc, ident[:])
            ones_c = sbp.tile([P, 1], F32, tag="ones_c")
            nc.vector.memset(ones_c[:], 1.0)
            ones_r = sbp.tile([1, P], F32, tag="ones_r")
            nc.vector.memset(ones_r[:], 1.0)
            ones_rb = sbp.tile([1, P], DT, tag="ones_rb")
            nc.vector.memset(ones_rb[:], 1.0)

            idx_sb = sbp.tile([P, totch], mybir.dt.int32, tag="idx")
            nc.sync.dma_start(out=idx_sb[:],
                              in_=idx_d[:].rearrange("(c p) -> p c", p=P))

            bas_sb, root_sb = [], []
            for l in range(3):
                bt = sbp.tile([P, NB * P], DT, tag=f"bas{l}")
                nc.sync.dma_start(out=bt[:], in_=bas_d[l][:])
                bas_sb.append(bt)
                rt = sbp.tile([P, P], DT, tag=f"root{l}")
                nc.sync.dma_start(out=rt[:], in_=root_d[l][:])
                root_sb.append(rt)
            gb_sb = []
            for l in range(2):
                gt = sbp.tile([1, 2 * H], F32, tag=f"gb{l}")
                nc.sync.dma_start(out=gt[:], in_=gb_d[l][:])
                gb_sb.append(gt)
            bias3_sb = sbp.tile([1, OUT], DT, tag="bias3")
            nc.sync.dma_start(out=bias3_sb[:], in_=bias3_d[:])

            hT = [sbp.tile([P, NLOCP], DT, tag=f"hT{l}", name=f"hT{l}")
                  for l in range(3)]
            nc.sync.dma_start(out=hT[0][:IN, :], in_=xT_d[:])

            allg_in = [drp.tile([NLOCP, P], DT, tag=f"agin{l}", name=f"agin{l}")
                       for l in range(2)]
            tabs = [None,
                    drp.tile([NPAD, P], DT, tag="tab2", name="tab2"),
                    drp.tile([NPAD, P], DT, tag="tab3", name="tab3")]
            st_in = [drp.tile([1, 2 * H], F32, tag=f"sti{l}", name=f"sti{l}")
                     for l in range(2)]
            st_out = [drp.tile([1, 2 * H], F32, tag=f"sto{l}", name=f"sto{l}")
                      for l in range(2)]

            for l in range(3):
                last = l == 2
                iw, ow = inW[l], outW[l]
                tab_ap = xtab[:] if l == 0 else tabs[l][:]
                slab = sbp.tile([P, NT * ow], F32, tag=f"slab{l}")
                if not last:
                    stats = psS.tile([1, ow], F32, tag="st")
                    stats2 = psS.tile([1, ow], F32, tag="st2")
                accT = None
                for w in range(NWIN):
                    q0, qn = int(base[w]), quota[w]
                    acc = psA.tile([P, SELW], F32, tag="acc")
                    selw = sbl.tile([P, qn * SELW], DT, tag="sel")
                    nc.sync.dma_start(
                        out=selw[:].rearrange("p (c s) -> p c s", s=SELW),
                        in_=sel_d[l][q0:q0 + qn].rearrange("c p s -> p c s"))
                    for q in range(qn):
                        gt = sbl.tile([P, P], DT, tag="g")
                        nc.gpsimd.indirect_dma_start(
                            out=gt[:], out_offset=None, in_=tab_ap,
                            in_offset=bass.IndirectOffsetOnAxis(
                                ap=idx_sb[:, q0 + q:q0 + q + 1], axis=0))
                        nc.tensor.matmul(
                            out=acc[:], lhsT=gt[:],
                            rhs=selw[:, q * SELW:(q + 1) * SELW],
                            start=(q == 0), stop=(q == qn - 1))
                    if w % 2 == 0:
                        accT = sbl.tile([P, NB, P], DT, tag="accT")
                    half = (w % 2) * WIN
                    nc.vector.tensor_copy(
                        out=accT[:, :, half:half + WIN],
                        in_=acc[:].rearrange("p (b n) -> p b n", b=NB))
                    if w % 2 == 1:
                        t = w // 2
                        ot = psB.tile([P, ow], F32, tag="ot")
                        for b in range(NB):
                            nc.tensor.matmul(
                                out=ot[:], lhsT=accT[:iw, b, :],
                                rhs=bas_sb[l][:iw, b * P:b * P + ow],
                                start=(b == 0), stop=False)
                        nc.tensor.matmul(
                            out=ot[:], lhsT=hT[l][:iw, t * P:(t + 1) * P],
                            rhs=root_sb[l][:iw, :ow],
                            start=False, stop=last)
                        if last:
                            nc.tensor.matmul(
                                out=ot[:], lhsT=ones_rb[:, :P],
                                rhs=bias3_sb[:], start=False, stop=True)
                        sl = slab[:, t * ow:(t + 1) * ow]
                        nc.vector.tensor_copy(out=sl, in_=ot[:])
                        if DBG == "conv1" and l == 0:
                            nc.sync.dma_start(out=out_d[t * P:(t + 1) * P, :],
                                              in_=sl[:, :OUT])
                        if not last:
                            sq = sbl.tile([P, ow], F32, tag="sq")
                            nc.vector.tensor_mul(out=sq[:], in0=sl, in1=sl)
                            s1s = sbl.tile([P, ow], F32, tag="s1s")
                            nc.vector.tensor_copy(out=s1s[:], in_=sl)
                            kp = P if t < NT - 1 else NLOC - (NT - 1) * P
                            nc.tensor.matmul(
                                out=stats[:], lhsT=ones_c[:kp, :],
                                rhs=s1s[:kp, :],
                                start=(t == 0), stop=(t == NT - 1))
                            nc.tensor.matmul(
                                out=stats2[:], lhsT=ones_c[:kp, :],
                                rhs=sq[:kp, :], start=(t == 0),
                                stop=(t == NT - 1))
                if not last:
                    st_sb = sbl.tile([1, 2 * ow], F32, tag="stsb")
                    nc.vector.tensor_copy(out=st_sb[:, :ow], in_=stats[:])
                    nc.vector.tensor_copy(out=st_sb[:, ow:], in_=stats2[:])
                    nc.sync.dma_start(out=st_in[l][:], in_=st_sb[:])
                    nc.gpsimd.collective_compute(
                        "AllReduce", mybir.AluOpType.add,
                        replica_groups=[list(range(NCORE))],
                        ins=[st_in[l].opt()], outs=[st_out[l].opt()])
                    stg = sbl.tile([1, 2 * ow], F32, tag="stg")
                    nc.sync.dma_start(out=stg[:], in_=st_out[l][:])
                    if DBG == f"stats{l + 1}":
                        nc.sync.dma_start(out=out_d[0:1, :],
                                          in_=stg[:, 0:OUT])
                        nc.sync.dma_start(out=out_d[1:2, :],
                                          in_=stg[:, ow:ow + OUT])
                        nc.sync.dma_start(out=out_d[2:3, :],
                                          in_=st_sb[:, 0:OUT])
                        nc.sync.dma_start(out=out_d[3:4, :],
                                          in_=st_sb[:, ow:ow + OUT])
                    # scale/shift rows (distinct tiles; avoid slice aliasing)
                    scsh = sbl.tile([1, 2 * ow], F32, tag="scsh")
                    mean_t = sbl.tile([1, ow], F32, tag="bn_mean")
                    tmp = sbl.tile([1, ow], F32, tag="bn_tmp")
                    mean2 = sbl.tile([1, ow], F32, tag="bn_m2")
                    sc_t = sbl.tile([1, ow], F32, tag="bn_sc")
                    ms_t = sbl.tile([1, ow], F32, tag="bn_ms")
                    sh_t = sbl.tile([1, ow], F32, tag="bn_sh")
                    nc.vector.tensor_scalar_mul(out=mean_t[:], in0=stg[:, :ow],
                                                scalar1=1.0 / N)
                    nc.vector.tensor_scalar_mul(out=tmp[:], in0=stg[:, ow:],
                                                scalar1=1.0 / N)
                    nc.vector.tensor_mul(out=mean2[:], in0=mean_t[:],
                                         in1=mean_t[:])
                    nc.vector.tensor_sub(out=tmp[:], in0=tmp[:], in1=mean2[:])
                    nc.vector.tensor_scalar_add(out=tmp[:], in0=tmp[:],
                                                scalar1=BN_EPS)
                    nc.scalar.activation(out=tmp[:], in_=tmp[:],
                                         func=mybir.ActivationFunctionType.Sqrt)
                    nc.vector.reciprocal(out=tmp[:], in_=tmp[:])
                    nc.vector.tensor_mul(out=sc_t[:], in0=tmp[:],
                                         in1=gb_sb[l][:, :ow])
                    nc.vector.tensor_mul(out=ms_t[:], in0=mean_t[:],
                                         in1=sc_t[:])
                    nc.vector.tensor_sub(out=sh_t[:], in0=gb_sb[l][:, ow:],
                                         in1=ms_t[:])
                    nc.vector.tensor_copy(out=scsh[:, :ow], in_=sc_t[:])
                    nc.vector.tensor_copy(out=scsh[:, ow:], in_=sh_t[:])
                    bc = psC.tile([P, 2 * ow], F32, tag="bc")
                    nc.tensor.matmul(out=bc[:], lhsT=ones_r[:, :P],
                                     rhs=scsh[:], start=True, stop=True)
                    bcs = sbl.tile([P, 2 * ow], F32, tag="bcs")
                    nc.vector.tensor_copy(out=bcs[:], in_=bc[:])
                    if DBG == f"bn{l + 1}":
                        nc.sync.dma_start(out=out_d[0:P, :],
                                          in_=bcs[:, :OUT])
                        nc.sync.dma_start(out=out_d[P:2 * P, :],
                                          in_=bcs[:, ow:ow + OUT])
                    for t in range(NT):
                        sl = slab[:, t * ow:(t + 1) * ow]
                        nc.vector.tensor_mul(out=sl, in0=sl, in1=bcs[:, :ow])
                        nc.vector.tensor_add(out=sl, in0=sl, in1=bcs[:, ow:])
                        lk = sbl.tile([P, ow], F32, tag="lk")
                        nc.vector.tensor_scalar_mul(out=lk[:], in0=sl,
                                                    scalar1=0.1)
                        nc.vector.tensor_tensor(out=sl, in0=sl, in1=lk[:],
                                                op=mybir.AluOpType.max)
                        if DBG == f"h{l + 1}":
                            nc.sync.dma_start(out=out_d[t * P:(t + 1) * P, :],
                                              in_=sl[:, :OUT])
                        hbf = sbl.tile([P, ow], DT, tag="hbf")
                        nc.vector.tensor_copy(out=hbf[:], in_=sl)
                        nc.sync.dma_start(
                            out=allg_in[l][t * P:(t + 1) * P, :ow],
                            in_=hbf[:])
                        pt = psC.tile([P, P], DT, tag="pt")
                        nc.tensor.transpose(out=pt[:, :ow]
                                            if ow < P else pt[:],
                                            in_=hbf[:], identity=ident[:])
                        nc.vector.tensor_copy(
                            out=hT[l + 1][:ow, t * P:(t + 1) * P],
                            in_=pt[:ow, :P])
                    nc.gpsimd.collective_compute(
                        "AllGather", mybir.AluOpType.bypass,
                        replica_groups=[list(range(NCORE))],
                        ins=[allg_in[l].opt()], outs=[tabs[l + 1].opt()])
                else:
                    for t in range(NT):
                        sl = slab[:, t * ow:(t + 1) * ow]
                        sq = sbl.tile([P, ow], F32, tag="sq")
                        nc.vector.tensor_mul(out=sq[:], in0=sl, in1=sl)
                        rs = sbl.tile([P, 1], F32, tag="rs")
                        nc.vector.tensor_reduce(
                            rs[:], sq[:], mybir.AxisListType.X,
                            mybir.AluOpType.add)
                        nc.scalar.activation(
                            out=rs[:], in_=rs[:],
                            func=mybir.ActivationFunctionType.Sqrt)
                        nc.vector.tensor_scalar_max(out=rs[:], in0=rs[:],
                                                    scalar1=1e-12)
                        nc.vector.reciprocal(out=rs[:], in_=rs[:])
                        fin = sbl.tile([P, ow], F32, tag="fin")
                        nc.vector.tensor_tensor(
                            out=fin[:], in0=sl,
                            in1=rs[:].to_broadcast([P, ow]),
                            op=mybir.AluOpType.mult)
                        if DBG is None:
                            nc.sync.dma_start(
                                out=out_d[t * P:(t + 1) * P, :], in_=fin[:])
    nc.compile()
    return nc


def make_inputs(inputs, quota, totch, per_core):
    """Build per-core in_maps from the reference inputs."""
    x = np.asarray(inputs["x"], np.float32)
    xtab = np.zeros((NPAD, P), dtype=NPDT)
    for c in range(NCORE):
        xtab[c * NLOCP:c * NLOCP + NLOC, :IN] = x[c * NLOC:(c + 1) * NLOC]
    wts = {}
    for l, (nb, nio) in enumerate((("bases1", (IN, H)), ("bases2", (H, H)),
                                   ("bases3", (H, OUT)))):
        b = np.asarray(inputs[nb], np.float32)          # [NB, in, out]
        bt = np.zeros((P, NB * P), dtype=NPDT)
        for k in range(NB):
            bt[:nio[0], k * P:k * P + nio[1]] = b[k]
        wts[f"bases{l + 1}"] = bt
        r = np.asarray(inputs[f"root{l + 1}"], np.float32)
        rt = np.zeros((P, P), dtype=NPDT)
        rt[:nio[0], :nio[1]] = r
        wts[f"root{l + 1}"] = rt
    for l in (1, 2):
        wts[f"gb{l}"] = np.concatenate(
            [np.asarray(inputs[f"g{l}"], np.float32),
             np.asarray(inputs[f"b{l}"], np.float32)])[None, :]
    wts["bias3"] = np.asarray(inputs["bias3"], np.float32).astype(NPDT)[None, :]

    in_maps = []
    for c in range(NCORE):
        m = {"xtab": xtab,
             "xT": np.ascontiguousarray(
                 np.pad(x[c * NLOC:(c + 1) * NLOC],
                        ((0, NLOCP - NLOC), (0, 0))).T).astype(NPDT),
             "idx": per_core[c]["idx"]}
        for l in (1, 2, 3):
            m[f"sel{l}"] = per_core[c][f"sel{l}"]
        m.update(wts)
        in_maps.append(m)
    return in_maps


_CACHE = {}


def kernel(**inputs) -> np.ndarray:
    comps = [np.asarray(inputs[f"comp{l}"], np.float32) for l in (1, 2, 3)]
    quota, totch, per_core = host_prep(inputs["edge_index"],
                                       inputs["edge_type"], comps)
    key = tuple(quota)
    if key not in _CACHE:
        _CACHE[key] = build_program(quota, totch)
    nc = _CACHE[key]
    from concourse.bass2jax import run_bass_via_pjrt
    in_maps = make_inputs(inputs, quota, totch, per_core)
    res = run_bass_via_pjrt(nc, in_maps, n_cores=NCORE)
    out = np.concatenate([res[c]["out"][:NLOC] for c in range(NCORE)], axis=0)
    return out.astype(np.float32)

